# revision 14
# baseline (speedup 1.0000x reference)
"""Trainium2 Bass kernel for nn_CrossAttentionModule (cross-attention transformer
block). Self-contained: accepts FULL inputs, shards across 8 NeuronCores
internally (core c -> batch c//2, T-half c%2), returns FULL output.

v2: minimizes host->device bytes per call. Everything shipped is bf16; the six
weight matrices are sharded 1/8-per-core and AllGathered on device; context is
split in half per core pair (core ships its S-half, computes K/V for it, and
the pair exchanges K/V via a pair AllGather). Shipped bytes: ~56MB vs ~480MB
for the replicated-f32 layout.

Layout: activations feature-major (D on partitions, tokens free), weights
pre-transposed host-side to [in, out]. Matmul operands bf16, PSUM f32,
residual stream f32.
"""

import sys

sys.path.insert(0, "/opt/trn_rl_repo")

import numpy as np
import concourse.bass as bass
import concourse.mybir as mybir
import concourse.tile as tile
from concourse import bacc
from concourse.bass_utils import run_bass_kernel_spmd

P = 128
EPS = 1e-5
F32 = mybir.dt.float32
F32R = mybir.dt.float32r
BF16 = mybir.dt.bfloat16
AF = mybir.ActivationFunctionType
OP = mybir.AluOpType

_CACHE = {}
_last_in_maps = None


def _layer_norm(nc, tc, ctx_pools, src, dst, g_t, b_t, KD, W, uid=""):
    """LN over the partition-tiled feature dim.

    src/dst: SBUF tiles [P, KD, W]. g_t/b_t: [P, KD] fp32 scale/shift.
    Stats via all-ones matmul (sums broadcast to all 128 partitions), apply on
    DVE. Processes W in chunks of <=1024 columns. ones_s must match src dtype;
    ones_f is f32r for the squared-sum matmul.
    """
    ones_s, ones_f, eps_t = ctx_pools
    CH = 1024 if W % 1024 == 0 else W
    assert W % CH == 0
    with (
        tc.tile_pool(name=f"lnps{uid}", bufs=1, space="PSUM") as stats_ps,
        tc.tile_pool(name=f"lnpipe{uid}", bufs=2) as pipe,
        tc.tile_pool(name=f"lnone{uid}", bufs=1) as one,
    ):
        for c0 in range(0, W, CH):
            ssum = stats_ps.tile([P, CH], F32, tag="ssum")
            ssq = stats_ps.tile([P, CH], F32, tag="ssq")
            for j in range(KD):
                sq = pipe.tile([P, CH], F32R, tag="lnsq")
                nc.vector.tensor_mul(
                    sq, src[:, j, c0 : c0 + CH], src[:, j, c0 : c0 + CH]
                )
                for n0 in range(0, CH, 512):
                    nc.tensor.matmul(
                        ssum[:, n0 : n0 + 512],
                        lhsT=ones_s,
                        rhs=src[:, j, c0 + n0 : c0 + n0 + 512],
                        start=(j == 0),
                        stop=(j == KD - 1),
                    )
                    nc.tensor.matmul(
                        ssq[:, n0 : n0 + 512],
                        lhsT=ones_f,
                        rhs=sq[:, n0 : n0 + 512],
                        start=(j == 0),
                        stop=(j == KD - 1),
                    )
            D = KD * P
            mu = one.tile([P, CH], F32, tag="lnmu")
            nc.scalar.activation(mu, ssum, AF.Copy, scale=1.0 / D)
            r = one.tile([P, CH], F32, tag="lnr")
            nc.vector.tensor_mul(r, mu, mu)
            w = one.tile([P, CH], F32, tag="lnw")
            nc.scalar.activation(w, ssq, AF.Copy, scale=1.0 / D)
            nc.vector.tensor_tensor(out=w, in0=w, in1=r, op=OP.subtract)
            nc.scalar.activation(w, w, AF.Sqrt, bias=eps_t)
            nc.vector.reciprocal(r, w)
            for j in range(KD):
                t0 = pipe.tile([P, CH], F32, tag="lnsq")
                nc.vector.tensor_tensor(
                    out=t0, in0=src[:, j, c0 : c0 + CH], in1=mu, op=OP.subtract
                )
                nc.vector.tensor_tensor(out=t0, in0=t0, in1=r, op=OP.mult)
                nc.vector.tensor_scalar(
                    out=dst[:, j, c0 : c0 + CH],
                    in0=t0,
                    scalar1=g_t[:, j : j + 1],
                    scalar2=b_t[:, j : j + 1],
                    op0=OP.mult,
                    op1=OP.add,
                )


def _build_nc(T, S, D, DFF, H, mock_cc=False):
    """Build + compile the per-core Bass program (SPMD; identical all cores).

    T: per-core token count (T_full//2). S: full context length; each core
    ships S//2 context tokens and exchanges K/V with its pair peer.
    mock_cc replaces collectives with local-block DMAs (single-core timing sim
    only -- numerically wrong across cores).
    """
    KD = D // P  # feature k-tiles
    SH = S // 2  # per-core context tokens
    STH = SH // P  # local context s-tiles
    ST = S // P  # full context s-tiles
    MO = DFF // P  # ffn hidden tiles
    NPAIR = H // 2
    DH = D // H
    assert DH == 64 and KD == NPAIR

    nc = bacc.Bacc("TRN2", target_bir_lowering=False, debug=False, num_devices=8)

    # --- external inputs (bf16 except gb) ---
    xT = nc.dram_tensor("xT", [D, T], BF16, kind="ExternalInput")
    ctxT = nc.dram_tensor("ctxT", [D, SH], BF16, kind="ExternalInput")
    # weight shards: wqkv = rows of concat([wqT,wkT,wvT]); wow12 = flat chunk
    # of concat([woT, w1T, w2T]) raveled.
    NQKV = 3 * D * D
    NOW12 = D * D + D * DFF + DFF * D
    wqkv_sh = nc.dram_tensor("wqkv_sh", [NQKV // 8], BF16, kind="ExternalInput")
    wow12_sh = nc.dram_tensor("wow12_sh", [NOW12 // 8], BF16, kind="ExternalInput")
    gb = nc.dram_tensor("gb", [6, D], F32, kind="ExternalInput")
    outT = nc.dram_tensor("outT", [D, T], BF16, kind="ExternalOutput")

    # --- internal DRAM: collective bounces + gathered tensors ---
    wqkv_b = nc.dram_tensor("wqkv_b", [NQKV // 8], BF16, kind="Internal")
    wo_b = nc.dram_tensor("wo_b", [D * D // 8], BF16, kind="Internal")
    w1_b = nc.dram_tensor("w1_b", [D * DFF // 8], BF16, kind="Internal")
    w2_b = nc.dram_tensor("w2_b", [DFF * D // 8], BF16, kind="Internal")
    wqkvF = nc.dram_tensor(
        "wqkvF", [NQKV], BF16, kind="Internal", addr_space="Shared"
    )
    woF = nc.dram_tensor("woF", [D * D], BF16, kind="Internal", addr_space="Shared")
    w1F = nc.dram_tensor(
        "w1F", [D * DFF], BF16, kind="Internal", addr_space="Shared"
    )
    w2F = nc.dram_tensor(
        "w2F", [DFF * D], BF16, kind="Internal", addr_space="Shared"
    )
    wq_v = wqkvF[:][0 : D * D].rearrange("(k p m) -> p k m", p=P, m=D)
    wk_v = wqkvF[:][D * D : 2 * D * D].rearrange("(k p m) -> p k m", p=P, m=D)
    wv_v = wqkvF[:][2 * D * D : 3 * D * D].rearrange("(k p m) -> p k m", p=P, m=D)
    wo_v = woF[:].rearrange("(k p m) -> p k m", p=P, m=D)
    w1_v = w1F[:].rearrange("(k p m) -> p k m", p=P, m=DFF)
    w2_v = w2F[:].rearrange("(k p m) -> p k m", p=P, m=D)
    # K/V packed for one pair exchange: block 0 = K (k, p, s) feature-major,
    # block 1 = V token-major (si, p, (h dh)); both are KD*P*SH elements.
    kvloc = nc.dram_tensor("kvloc", [2, KD, P, SH], BF16, kind="Internal")
    kvAll = nc.dram_tensor("kvAll", [2, 2, KD, P, SH], BF16, kind="Internal")

    xT_r = xT[:].rearrange("(k p) t -> p k t", p=P)
    ctxT_r = ctxT[:].rearrange("(k p) t -> p k t", p=P)
    gb_r = gb[:].rearrange("g (k p) -> g p k", p=P)
    outT_r = outT[:].rearrange("(k p) t -> p k t", p=P)

    GROUP_ALL = [list(range(8))]
    GROUP_PAIR = [[0, 1], [2, 3], [4, 5], [6, 7]]

    def allgather(full, local, groups):
        if mock_cc:
            # timing mock: local block copy only
            blk = int(np.prod(full.shape[1:]))
            nblk = int(np.prod(local.shape)) // blk
            nc.gpsimd.dma_start(out=full[:][0:nblk], in_=local[:])
        else:
            nc.gpsimd.collective_compute(
                "AllGather",
                mybir.AluOpType.bypass,
                replica_groups=groups,
                ins=[local[:]],
                outs=[full[:]],
            )

    TC = T // 512  # 512-wide t-chunks

    with tile.TileContext(nc) as tc:
        from contextlib import ExitStack

        with ExitStack() as root:
            root.enter_context(
                nc.allow_low_precision(reason="bf16 matmul operands by design")
            )
            consts = root.enter_context(tc.tile_pool(name="consts", bufs=1))
            ones32 = consts.tile([P, P], F32)
            nc.vector.memset(ones32, 1.0)
            ones_f = consts.tile([P, P], F32R)
            nc.vector.tensor_copy(ones_f, ones32)
            ones_b = consts.tile([P, P], BF16)
            nc.vector.tensor_copy(ones_b, ones32)
            gbt = consts.tile([P, 6, KD], F32)
            for g in range(6):
                nc.sync.dma_start(out=gbt[:, g, :], in_=gb_r[g])
            eps_t = consts.tile([P, 1], F32)
            nc.vector.memset(eps_t, EPS)

            # --- weight shard bounces (sync queue) + gathers (gpsimd, in order)
            nc.sync.dma_start(out=wqkv_b[:], in_=wqkv_sh[:])
            NWO = D * D // 8
            NW1 = D * DFF // 8
            nc.sync.dma_start(out=wo_b[:], in_=wow12_sh[:][0:NWO])
            nc.sync.dma_start(out=w1_b[:], in_=wow12_sh[:][NWO : NWO + NW1])
            nc.sync.dma_start(out=w2_b[:], in_=wow12_sh[:][NWO + NW1 :])
            allgather(wqkvF, wqkv_b, GROUP_ALL)

            q_ctx = tc.tile_pool(name="qp", bufs=1)
            q_pool = q_ctx.__enter__()
            Q = q_pool.tile([P, KD, T], BF16)

            xin = root.enter_context(tc.tile_pool(name="xin", bufs=1, side="right"))
            xt = xin.tile([P, KD, T], BF16)
            for j in range(KD):
                nc.sync.dma_start(out=xt[:, j, :], in_=xT_r[:, j, :])

            # ---------- phase 3-5: LN(ctx half) ; K,V local -> exchange ----------
            with ExitStack() as ph:
                cnp = ph.enter_context(tc.tile_pool(name="cnp", bufs=1, side="right"))
                cn = cnp.tile([P, KD, SH], BF16)
                with tc.tile_pool(name="cin", bufs=1, side="right") as cin2:
                    ct = cin2.tile([P, KD, SH], BF16)
                    for j in range(KD):
                        nc.sync.dma_start(out=ct[:, j, :], in_=ctxT_r[:, j, :])
                    _layer_norm(
                        nc, tc, (ones_b, ones_f, eps_t), ct, cn,
                        gbt[:, 2, :], gbt[:, 3, :], KD, SH, uid="b",
                    )
                # K rows (feature-major) per mo-tile -> kloc DRAM
                with (
                    tc.tile_pool(name="wst2", bufs=3, side="right") as wst,
                    tc.tile_pool(name="kst", bufs=2, side="right") as kst,
                    tc.tile_pool(name="mmpsk", bufs=3, space="PSUM") as mps,
                ):
                    WSP = min(512, D)
                    for sp in range(0, D, WSP):
                        wk_t = wst.tile([P, KD, WSP], BF16, tag="wk")
                        for k in range(KD):
                            nc.sync.dma_start(
                                out=wk_t[:, k, :],
                                in_=wk_v[:, k, sp : sp + WSP],
                            )
                        for mo_s in range(WSP // P):
                            mo = sp // P + mo_s
                            kstage = kst.tile([P, SH], BF16, tag="kstage")
                            for t0 in range(0, SH, 512):
                                ps = mps.tile([P, 512], F32, tag="kps")
                                for k in range(KD):
                                    nc.tensor.matmul(
                                        ps,
                                        lhsT=wk_t[:, k, mo_s * P : (mo_s + 1) * P],
                                        rhs=cn[:, k, t0 : t0 + 512],
                                        start=(k == 0),
                                        stop=(k == KD - 1),
                                    )
                                nc.vector.tensor_copy(kstage[:, t0 : t0 + 512], ps)
                            nc.gpsimd.dma_start(out=kvloc[:][0][mo], in_=kstage)
                # V token-major -> vloc DRAM
                with (
                    tc.tile_pool(name="wvp", bufs=1) as wvp,
                    tc.tile_pool(name="vst", bufs=1, side="right") as vsp,
                    tc.tile_pool(name="mmpsv", bufs=3, space="PSUM") as mps,
                ):
                    vst = vsp.tile([P, STH, H, DH], BF16)
                    DCH = min(512, D)
                    for dh in range(0, D, DCH):  # d-chunks
                        wv_t = wvp.tile([P, KD, DCH], BF16, tag="wv")
                        for k in range(KD):
                            nc.sync.dma_start(
                                out=wv_t[:, k, :], in_=wv_v[:, k, dh : dh + DCH]
                            )
                        for si in range(STH):
                            ps = mps.tile([P, DCH], F32, tag="vps")
                            for k in range(KD):
                                nc.tensor.matmul(
                                    ps,
                                    lhsT=cn[:, k, si * P : (si + 1) * P],
                                    rhs=wv_t[:, k, :],
                                    start=(k == 0),
                                    stop=(k == KD - 1),
                                )
                            h0 = dh // DH
                            nc.vector.tensor_copy(
                                vst[:, si, h0 : h0 + DCH // DH, :],
                                ps.rearrange("p (h d) -> p h d", d=DH),
                            )
                    for si in range(STH):
                        nc.gpsimd.dma_start(out=kvloc[:][1][si], in_=vst[:, si])

                # pair exchange, then remaining weight gather rides behind
                allgather(kvAll, kvloc, GROUP_PAIR)
                allgather(woF, wo_b, GROUP_ALL)
                allgather(w1F, w1_b, GROUP_ALL)
                allgather(w2F, w2_b, GROUP_ALL)

            # ---------- phase 1-2: LN(x) -> xn ; Q = Wq @ xn ----------
            with ExitStack() as ph:
                xnp = ph.enter_context(tc.tile_pool(name="xnp", bufs=1, side="right"))
                wst = ph.enter_context(tc.tile_pool(name="wst", bufs=3))
                mps = ph.enter_context(tc.tile_pool(name="mmps", bufs=4, space="PSUM"))

                xn = xnp.tile([P, KD, T], BF16)
                _layer_norm(
                    nc, tc, (ones_b, ones_f, eps_t), xt, xn,
                    gbt[:, 0, :], gbt[:, 1, :], KD, T, uid="a",
                )
                WSP = min(512, D)
                for sp in range(0, D, WSP):
                    wq_t = wst.tile([P, KD, WSP], BF16, tag="wq")
                    for k in range(KD):
                        nc.sync.dma_start(
                            out=wq_t[:, k, :], in_=wq_v[:, k, sp : sp + WSP]
                        )
                    for mo_s in range(WSP // P):
                        mo = sp // P + mo_s
                        for t0 in range(0, T, 512):
                            ps = mps.tile([P, 512], F32, tag="qps")
                            for k in range(KD):
                                nc.tensor.matmul(
                                    ps,
                                    lhsT=wq_t[:, k, mo_s * P : (mo_s + 1) * P],
                                    rhs=xn[:, k, t0 : t0 + 512],
                                    start=(k == 0),
                                    stop=(k == KD - 1),
                                )
                            nc.vector.tensor_copy(Q[:, mo, t0 : t0 + 512], ps)

            # ---------- phase 6: attention ----------
            o_ctx = tc.tile_pool(name="op", bufs=1, side="right")
            o_pool = o_ctx.__enter__()
            O_all = o_pool.tile([P, KD, T], BF16)

            with ExitStack() as ph:
                vp_ = ph.enter_context(tc.tile_pool(name="vp", bufs=1))
                Vp = vp_.tile([P, ST, H, DH + 1], BF16)
                nc.vector.tensor_copy(
                    Vp.rearrange("p a b c -> p (a b) c")[:, :, DH : DH + 1],
                    ones_b[:, 0:1, None].to_broadcast((P, ST * H, 1)),
                )
                for r in range(2):
                    for si in range(STH):
                        nc.sync.dma_start(
                            out=Vp[:, r * STH + si, :, 0:DH], in_=kvAll[:][r][1][si].rearrange("p (h d) -> p h d", d=DH)
                        )

                kin = ph.enter_context(tc.tile_pool(name="kin", bufs=2))
                pts = ph.enter_context(tc.tile_pool(name="pts", bufs=3))
                sps_ = ph.enter_context(tc.tile_pool(name="sps", bufs=2, space="PSUM"))
                ops_ = ph.enter_context(tc.tile_pool(name="ops", bufs=1, space="PSUM"))
                rps = ph.enter_context(tc.tile_pool(name="rps", bufs=1, space="PSUM"))
                rtmp = ph.enter_context(tc.tile_pool(name="rtmp", bufs=2))
                osh = ph.enter_context(tc.tile_pool(name="osh", bufs=2))

                for pair in range(NPAIR):
                    kp = kin.tile([P, S], BF16, tag="kp")
                    for r in range(2):
                        nc.sync.dma_start(
                            out=kp[:, r * SH : (r + 1) * SH], in_=kvAll[:][r][0][pair]
                        )
                    he, ho = 2 * pair, 2 * pair + 1
                    for t0 in range(0, T, 512):
                        pse = ops_.tile([P, 512], F32, tag="pse")
                        pso = ops_.tile([P, 512], F32, tag="pso")
                        for si in range(ST):
                            se = sps_.tile([P, 512], F32, tag="se")
                            so = sps_.tile([P, 512], F32, tag="so")
                            nc.tensor.matmul(
                                se,
                                lhsT=kp[0:64, si * P : (si + 1) * P],
                                rhs=Q[0:64, pair, t0 : t0 + 512],
                                start=True, stop=True,
                            )
                            nc.tensor.matmul(
                                so,
                                lhsT=kp[64:128, si * P : (si + 1) * P],
                                rhs=Q[64:128, pair, t0 : t0 + 512],
                                start=True, stop=True,
                            )
                            pe = pts.tile([P, 512], BF16, tag="pe")
                            po = pts.tile([P, 512], BF16, tag="po")
                            nc.scalar.activation(pe, se, AF.Exp, scale=0.125)
                            nc.scalar.activation(po, so, AF.Exp, scale=0.125)
                            nc.tensor.matmul(
                                pse[0:65, :],
                                lhsT=Vp[:, si, he, :],
                                rhs=pe,
                                start=(si == 0), stop=(si == ST - 1),
                            )
                            nc.tensor.matmul(
                                pso[0:65, :],
                                lhsT=Vp[:, si, ho, :],
                                rhs=po,
                                start=(si == 0), stop=(si == ST - 1),
                            )
                        # normalize rows 0:64 by row 64 (sums): recip on p64,
                        # K=1 matmul with ones broadcasts it to rows 0:64.
                        re = rtmp.tile([P, 512], F32R, tag="re")
                        re2 = rtmp.tile([P, 512], F32R, tag="re2")
                        nc.vector.reciprocal(re[64:65, :], pse[64:65, :])
                        nc.vector.reciprocal(re2[64:65, :], pso[64:65, :])
                        rbe = rps.tile([64, 512], F32, tag="rbe")
                        rbo = rps.tile([64, 512], F32, tag="rbo")
                        nc.tensor.matmul(
                            rbe, lhsT=ones_f[64:65, 0:64], rhs=re[64:65, :],
                            start=True, stop=True,
                        )
                        nc.tensor.matmul(
                            rbo, lhsT=ones_f[64:65, 0:64], rhs=re2[64:65, :],
                            start=True, stop=True,
                        )
                        rbs = rtmp.tile([64, 512], F32, tag="rbs")
                        rbs2 = rtmp.tile([64, 512], F32, tag="rbs2")
                        nc.vector.tensor_copy(rbs, rbe)
                        nc.vector.tensor_copy(rbs2, rbo)
                        nc.vector.tensor_tensor(
                            out=O_all[0:64, pair, t0 : t0 + 512],
                            in0=pse[0:64, :], in1=rbs, op=OP.mult,
                        )
                        ot = osh.tile([64, 512], BF16, tag="ot")
                        nc.vector.tensor_tensor(
                            out=ot, in0=pso[0:64, :], in1=rbs2, op=OP.mult,
                        )
                        nc.gpsimd.dma_start(
                            out=O_all[64:128, pair, t0 : t0 + 512], in_=ot
                        )

            q_ctx.__exit__(None, None, None)

            # ---------- phase 7: out1 = x + Wo @ O_all ----------
            out1_pool = root.enter_context(tc.tile_pool(name="out1p", bufs=1))
            out1 = out1_pool.tile([P, KD, T], F32R)

            with ExitStack() as ph:
                wst = ph.enter_context(tc.tile_pool(name="wst3", bufs=3))
                mps = ph.enter_context(tc.tile_pool(name="mmps3", bufs=4, space="PSUM"))
                WSP = min(512, D)
                for sp in range(0, D, WSP):
                    wo_t = wst.tile([P, KD, WSP], BF16, tag="wo")
                    for k in range(KD):
                        nc.sync.dma_start(
                            out=wo_t[:, k, :], in_=wo_v[:, k, sp : sp + WSP]
                        )
                    for mo_s in range(WSP // P):
                        mo = sp // P + mo_s
                        for t0 in range(0, T, 512):
                            ps = mps.tile([P, 512], F32, tag="ops2")
                            for k in range(KD):
                                nc.tensor.matmul(
                                    ps,
                                    lhsT=wo_t[:, k, mo_s * P : (mo_s + 1) * P],
                                    rhs=O_all[:, k, t0 : t0 + 512],
                                    start=(k == 0),
                                    stop=(k == KD - 1),
                                )
                            nc.vector.tensor_tensor(
                                out=out1[:, mo, t0 : t0 + 512], in0=ps,
                                in1=xt[:, mo, t0 : t0 + 512], op=OP.add,
                            )

            o_ctx.__exit__(None, None, None)

            # ---------- phase 8: FFN ----------
            with ExitStack() as ph:
                hp = ph.enter_context(tc.tile_pool(name="hp", bufs=1))
                hT = hp.tile([P, KD, T], BF16)
                _layer_norm(
                    nc, tc, (ones_f, ones_f, eps_t), out1, hT,
                    gbt[:, 4, :], gbt[:, 5, :], KD, T, uid="c",
                )
                gp = ph.enter_context(tc.tile_pool(name="gp", bufs=1, side="right"))
                w1st = ph.enter_context(tc.tile_pool(name="w1st", bufs=1))
                w2st = ph.enter_context(tc.tile_pool(name="w2st", bufs=1))
                f1ps = ph.enter_context(tc.tile_pool(name="f1ps", bufs=2, space="PSUM"))
                f2ps = ph.enter_context(tc.tile_pool(name="f2ps", bufs=2, space="PSUM"))
                fst = ph.enter_context(tc.tile_pool(name="fst", bufs=2))
                TH = T // 2
                for th0 in range(0, T, TH):
                    gt = gp.tile([P, MO, TH], BF16, tag="gt")
                    WSP = min(512, DFF)
                    for sp in range(0, DFF, WSP):
                        w1_t = w1st.tile([P, KD, WSP], BF16, tag="w1")
                        for k in range(KD):
                            nc.sync.dma_start(
                                out=w1_t[:, k, :], in_=w1_v[:, k, sp : sp + WSP]
                            )
                        for mo_s in range(WSP // P):
                            mo = sp // P + mo_s
                            for t0 in range(0, TH, 512):
                                ps = f1ps.tile([P, 512], F32, tag="f1")
                                for k in range(KD):
                                    nc.tensor.matmul(
                                        ps,
                                        lhsT=w1_t[:, k, mo_s * P : (mo_s + 1) * P],
                                        rhs=hT[:, k, th0 + t0 : th0 + t0 + 512],
                                        start=(k == 0),
                                        stop=(k == KD - 1),
                                    )
                                nc.scalar.activation(
                                    gt[:, mo, t0 : t0 + 512], ps, AF.Gelu
                                )
                    DSP = min(256, D)
                    for sp in range(0, D, DSP):
                        w2_t = w2st.tile([P, MO, DSP], BF16, tag="w2")
                        for mo in range(MO):
                            nc.sync.dma_start(
                                out=w2_t[:, mo, :],
                                in_=w2_v[:, mo, sp : sp + DSP],
                            )
                        for do_s in range(DSP // P):
                            do = sp // P + do_s
                            for t0 in range(0, TH, 512):
                                ps = f2ps.tile([P, 512], F32, tag="f2")
                                for mo in range(MO):
                                    nc.tensor.matmul(
                                        ps,
                                        lhsT=w2_t[:, mo, do_s * P : (do_s + 1) * P],
                                        rhs=gt[:, mo, t0 : t0 + 512],
                                        start=(mo == 0),
                                        stop=(mo == MO - 1),
                                    )
                                fo = fst.tile([P, 512], BF16, tag="fo")
                                nc.vector.tensor_tensor(
                                    out=fo, in0=ps,
                                    in1=out1[:, do, th0 + t0 : th0 + t0 + 512],
                                    op=OP.add,
                                )
                                nc.gpsimd.dma_start(
                                    out=outT_r[:, do, th0 + t0 : th0 + t0 + 512],
                                    in_=fo,
                                )

    nc.compile()
    return nc


def _get_nc(T, S, D, DFF, H):
    key = (T, S, D, DFF, H)
    if key not in _CACHE:
        _CACHE[key] = _build_nc(T, S, D, DFF, H)
    return _CACHE[key]


def kernel(x, context, Wq, Wk, Wv, Wo, W1, W2, g1, b1, gc, bc, g2, b2):
    x = np.asarray(x, np.float32)
    context = np.asarray(context, np.float32)
    B, T, D = x.shape
    S = context.shape[1]
    SH = S // 2
    DFF = W1.shape[0]
    H = 16
    TL = T // 2  # per-core T slice
    nc = _get_nc(TL, S, D, DFF, H)

    bf = mybir.dt.np(BF16)
    wqT = np.asarray(Wq, np.float32).T.astype(bf)
    wkT = np.asarray(Wk, np.float32).T.astype(bf)
    wvT = np.asarray(Wv, np.float32).T.astype(bf)
    woT = np.asarray(Wo, np.float32).T.astype(bf)
    w1T = np.asarray(W1, np.float32).T.astype(bf)
    w2T = np.asarray(W2, np.float32).T.astype(bf)
    gbm = np.stack([
        np.asarray(v, np.float32) for v in (g1, b1, gc, bc, g2, b2)
    ])
    xb = np.asarray(x, np.float32).transpose(0, 2, 1).astype(bf)  # [B, D, T]
    cb = np.asarray(context, np.float32).transpose(0, 2, 1).astype(bf)
    wqkv = np.concatenate([wqT.ravel(), wkT.ravel(), wvT.ravel()])
    wof, w1f, w2f = woT.ravel(), w1T.ravel(), w2T.ravel()

    NQ = wqkv.size // 8
    NWO, NW1, NW2 = wof.size // 8, w1f.size // 8, w2f.size // 8
    in_maps = []
    for c in range(8):
        b, half = c // 2, c % 2
        in_maps.append({
            "xT": np.ascontiguousarray(xb[b, :, half * TL : (half + 1) * TL]),
            "ctxT": np.ascontiguousarray(cb[b, :, half * SH : (half + 1) * SH]),
            "wqkv_sh": wqkv[c * NQ : (c + 1) * NQ],
            "wow12_sh": np.concatenate([
                wof[c * NWO : (c + 1) * NWO],
                w1f[c * NW1 : (c + 1) * NW1],
                w2f[c * NW2 : (c + 1) * NW2],
            ]),
            "gb": gbm,
        })

    global _last_in_maps
    _last_in_maps = in_maps
    res = run_bass_kernel_spmd(nc, in_maps, core_ids=list(range(8)))
    out = np.empty((B, T, D), np.float32)
    for c in range(8):
        b, half = c // 2, c % 2
        out[b, half * TL : (half + 1) * TL, :] = res.results[c]["outT"].T.astype(np.float32)
    return out
